# revision 1
# baseline (speedup 1.0000x reference)
"""Trainium2 Bass kernel for nn_CR8_reg_cond_mul_6 (moe_routing).

Data-parallel over batch across 8 NeuronCores. Per core: 16 batches x 2048
tokens of a fused 1x1-conv chain + argmax routing + conditional matmuls.

Numerics strategy (fp32-equivalent where it matters):
- Classification chain (cl1/cl2/cl3): float32r (11-bit) hi/lo split, 3 matmuls
  per layer => ~2^-24 relative error (zero argmax flips vs fp32 reference).
  Biases added in fp32 on the ACT engine (or DVE for cls).
- Regression branch (reg1, CondMul1/2): single-pass float32r (1.5e-4 rel),
  error enters output scaled by 1/128 -> negligible.
- argmax selection: gpsimd partition_all_reduce(max) -> exact fp32 compare
  (is_equal) -> onehot; CondMul2 evaluated for ALL 128 classes via a
  block-structured [256,128] table (W3x) so the superclass/class gathers
  become plain matmuls; final result = ones^T @ (onehot * (reg_all + (b3+c)/128)).
"""

import numpy as np

import concourse.bass as bass
import concourse.bacc as bacc
import concourse.tile as tile
import concourse.mybir as mybir
import concourse.bass_isa as bass_isa
from concourse import bass_utils

F32 = mybir.dt.float32
F32R = mybir.dt.float32r

N_CORES = 8
B_FULL = 128
BS = B_FULL // N_CORES          # 16 batches per core
C = 128
W = 2048
T = 512                          # token tile (PSUM bank = 512 fp32)
NTILES = W // T                  # 4 tiles per batch
CLASSES = 128
SUPER = 8
SLOPE = 0.01


def _round_f32r(x):
    """Round fp32 array to 11 explicit mantissa bits (matches HW f32r rounding
    closely; exactness of hi+lo split does not depend on matching HW ties)."""
    x = np.ascontiguousarray(np.asarray(x, np.float32))
    xi = x.view(np.uint32)
    shift = np.uint32(12)  # 23 - 11
    half = np.uint32(1 << 11)
    mask = np.uint32(0xFFFFFFFF) << shift
    out = ((xi + half) & mask).view(np.float32).copy()
    out[~np.isfinite(x)] = x[~np.isfinite(x)]
    return out


def _split_f32r(x):
    hi = _round_f32r(x)
    lo = _round_f32r(np.asarray(x, np.float32) - hi)
    return hi, lo


def prepare_consts(cl1_w, cl1_b, cl2_w, cl2_b, cl3_w, cl3_b,
                   reg1_w, reg1_b, w2, b2, w3, b3):
    """Host-side packing of all weight tables. Returns dict name->np.float32
    array (f32r-destined arrays are pre-rounded to 11 bits)."""
    c = {}
    # conv lhsT layouts [K=c_in, M=c_out]
    for name, wmat in [("w1", cl1_w), ("w2c", cl2_w), ("c3", cl3_w[:CLASSES])]:
        hi, lo = _split_f32r(wmat.T)          # [128, 128]
        c[name + "hi"] = hi
        c[name + "lo"] = lo
    c["b1"] = cl1_b.astype(np.float32).reshape(128, 1)
    c["b2c"] = cl2_b.astype(np.float32).reshape(128, 1)
    c["b3c"] = cl3_b[:CLASSES].astype(np.float32).reshape(128, 1)
    # mask row, single f32r, as 4 column-selector variants [128, 4] so tile i
    # writes psum row i of a shared [4,T] accumulator (M=1 base-partition
    # restriction workaround); bias replicated per tile-row
    wm = _round_f32r(cl3_w[CLASSES:CLASSES + 1].T)               # [128,1]
    wm16s = np.zeros((128, 256), np.float32)
    ones16s = np.zeros((128, 256), np.float32)
    for sl in range(16):
        wm16s[:, 16 * sl + sl] = wm[:, 0]
        ones16s[:, 16 * sl + sl] = 1.0
    c["wm16s"] = wm16s
    c["ones16s"] = ones16s * np.float32(1.0 / CLASSES)  # folds the /128 scale
    c["bm16"] = np.full((16, 1), cl3_b[CLASSES], np.float32)     # [16,1] fp32
    # reg1: [K=128, M=128] single f32r; bias fp32 for ACT
    c["wr"] = _round_f32r(reg1_w.T)
    c["br"] = reg1_b.astype(np.float32).reshape(128, 1)
    # CondMul1 table: W2all[k, j] with k in [r(128); h1(128)], j = s*32+u,
    # packed for fp8 DoubleRow: lhsT[kp, i, m] = W2all[kp + 128*i, m]
    import ml_dtypes
    w2all = np.transpose(w2, (1, 0, 2)).reshape(256, 256).astype(np.float32)
    w2dr = np.stack([w2all[0:128], w2all[128:256]], axis=1)      # [128, 2, 256]
    c["w2dra"] = w2dr[:, :, 0:128].astype(ml_dtypes.float8_e4m3)
    c["w2drb"] = w2dr[:, :, 128:256].astype(ml_dtypes.float8_e4m3)
    b2all = b2.reshape(256).astype(np.float32)                   # j order
    c["b2a"] = b2all[0:128].reshape(128, 1)
    c["b2b"] = b2all[128:256].reshape(128, 1)
    # CondMul2 block table (UNSCALED; /128 lives in ones4s): fp8 DoubleRow
    # lhsT[jp, i, c] = W3x[jp + 128*i, c]
    w3x = np.zeros((256, CLASSES), np.float32)
    for cc in range(CLASSES):
        sc = cc // 16
        w3x[sc * 32:(sc + 1) * 32, cc] = w3[cc, :, 0]
    c["w3xdr"] = np.stack([w3x[0:128], w3x[128:256]], axis=1).astype(ml_dtypes.float8_e4m3)
    # (b3[c] + c) per-class fp32 scalar for the G op (unscaled; /128 in ones4s)
    c["b3iota"] = (b3[:, 0].astype(np.float64)
                   + np.arange(CLASSES)).astype(np.float32).reshape(128, 1)
    return c


CONST_SPECS = [
    # name, shape, kind: f32 | f32r | fp8
    ("w1hi", [128, 128], "f32r"), ("w1lo", [128, 128], "f32r"),
    ("w2chi", [128, 128], "f32r"), ("w2clo", [128, 128], "f32r"),
    ("c3hi", [128, 128], "f32r"), ("c3lo", [128, 128], "f32r"),
    ("b1", [128, 1], "f32"), ("b2c", [128, 1], "f32"), ("b3c", [128, 1], "f32"),
    ("wm16s", [128, 256], "f32r"), ("ones16s", [128, 256], "f32r"),
    ("bm16", [16, 1], "f32"),
    ("wr", [128, 128], "f32r"), ("br", [128, 1], "f32"),
    ("w2dra", [128, 2, 128], "fp8"), ("w2drb", [128, 2, 128], "fp8"),
    ("b2a", [128, 1], "f32"), ("b2b", [128, 1], "f32"),
    ("w3xdr", [128, 2, 128], "fp8"),
    ("b3iota", [128, 1], "f32"),
]


def build_nc(bs=BS):
    """Build the per-core Bass module (same NEFF for all 8 cores)."""
    nc = bacc.Bacc("TRN2", target_bir_lowering=False, debug=False)

    xhi_d = nc.dram_tensor("xhi", [bs, C, 1, W], F32, kind="ExternalInput")
    xlo_d = nc.dram_tensor("xlo", [bs, C, 1, W], F32, kind="ExternalInput")
    FP8 = mybir.dt.float8e4
    const_d = {}
    for name, shape, knd in CONST_SPECS:
        dt = FP8 if knd == "fp8" else F32
        const_d[name] = nc.dram_tensor(name, shape, dt, kind="ExternalInput")
    xr_d = nc.dram_tensor("x_real", [bs, 1, 1, W], F32, kind="ExternalOutput")
    mk_d = nc.dram_tensor("mask", [bs, 1, 1, W], F32, kind="ExternalOutput")

    with tile.TileContext(nc) as tc:
        with (
            tc.tile_pool(name="consts", bufs=1) as cp,
            tc.tile_pool(name="io", bufs=6) as io,
            tc.tile_pool(name="acts", bufs=6) as ap,
            tc.tile_pool(name="sel", bufs=6) as sp,
            tc.tile_pool(name="outs", bufs=3) as op_,
            tc.tile_pool(name="py", bufs=2, space="PSUM") as py,
            tc.tile_pool(name="pcr", bufs=2, space="PSUM") as pcr,
            tc.tile_pool(name="prh", bufs=2, space="PSUM") as prh,
            tc.tile_pool(name="pmx", bufs=1, space="PSUM") as pmx,
        ):
            # ---- load constants (f32r via DVE rounding copy; fp8 direct)
            cst = {}
            for name, shape, knd in CONST_SPECS:
                if knd == "fp8":
                    t = cp.tile(shape, FP8, tag=f"c_{name}")
                    nc.sync.dma_start(t[:], const_d[name].ap())
                    cst[name] = t
                    continue
                stage = cp.tile(shape, F32, tag=f"st_{name}")
                nc.sync.dma_start(stage[:], const_d[name].ap())
                if knd == "f32r":
                    t = cp.tile(shape, F32R, tag=f"c_{name}")
                    nc.vector.tensor_copy(t[:], stage[:])
                    cst[name] = t
                else:
                    cst[name] = stage

            xhv = xhi_d.ap().squeeze(2).bitcast(F32R)
            xlv = xlo_d.ap().squeeze(2).bitcast(F32R)
            assert bs % 4 == 0, "batch grouping assumes bs divisible by 4"
            xrv = (xr_d.ap().squeeze(2).squeeze(1)
                   .rearrange("(g four) (n t) -> g (four n) t", four=4, t=T))
            mkv = (mk_d.ap().squeeze(2).squeeze(1)
                   .rearrange("(g four) (n t) -> g (four n) t", four=4, t=T))

            for b in range(bs):
                # 4-batch psum accumulators: 16 mask rows / 16 x_real rows
                if b % 4 == 0:
                    pm_t = pmx.tile([16, T], F32, tag="pmask")
                    px_t = pmx.tile([16, T], F32, tag="pxr")
                    pm = pm_t[:]
                    px = px_t[:]
                for i in range(NTILES):
                    # ---- load pre-split x tiles [128, 512] (host rounds)
                    xhi = io.tile([128, T], F32R, tag="xhi")
                    nc.sync.dma_start(xhi[:], xhv[b, :, bass.ts(i, T)])
                    xlo = io.tile([128, T], F32R, tag="xlo")
                    nc.scalar.dma_start(xlo[:], xlv[b, :, bass.ts(i, T)])
                    # ---- L1: y1 = W1 @ x (3-term f32r split), bias via ACT
                    y1 = py.tile([128, T], F32, tag="y")
                    nc.tensor.matmul(y1[:], cst["w1hi"][:], xhi[:], start=True, stop=False)
                    nc.tensor.matmul(y1[:], cst["w1lo"][:], xhi[:], start=False, stop=False)
                    nc.tensor.matmul(y1[:], cst["w1hi"][:], xlo[:], start=False, stop=True)
                    h1f = ap.tile([128, T], F32, tag="h1f")
                    nc.scalar.activation(h1f[:], y1[:], mybir.ActivationFunctionType.Lrelu,
                                         bias=cst["b1"][:], scale=1.0, alpha=SLOPE)
                    h1hi = ap.tile([128, T], F32R, tag="h1hi")
                    nc.gpsimd.tensor_copy(h1hi[:], h1f[:])
                    h1lo = ap.tile([128, T], F32R, tag="h1lo")
                    nc.vector.tensor_tensor(h1lo[:], h1f[:], h1hi[:].bitcast(F32),
                                            op=mybir.AluOpType.subtract)
                    # ---- L2
                    y2 = py.tile([128, T], F32, tag="y")
                    nc.tensor.matmul(y2[:], cst["w2chi"][:], h1hi[:], start=True, stop=False)
                    nc.tensor.matmul(y2[:], cst["w2clo"][:], h1hi[:], start=False, stop=False)
                    nc.tensor.matmul(y2[:], cst["w2chi"][:], h1lo[:], start=False, stop=True)
                    x2f = ap.tile([128, T], F32, tag="x2f")
                    nc.scalar.activation(x2f[:], y2[:], mybir.ActivationFunctionType.Lrelu,
                                         bias=cst["b2c"][:], scale=1.0, alpha=SLOPE)
                    x2hi = ap.tile([128, T], F32R, tag="x2hi")
                    nc.gpsimd.tensor_copy(x2hi[:], x2f[:])
                    x2lo = ap.tile([128, T], F32R, tag="x2lo")
                    nc.vector.tensor_tensor(x2lo[:], x2f[:], x2hi[:].bitcast(F32),
                                            op=mybir.AluOpType.subtract)
                    # ---- L3: cls scores (no activation)
                    ycls = pcr.tile([128, T], F32, tag="cr")
                    nc.tensor.matmul(ycls[:], cst["c3hi"][:], x2hi[:], start=True, stop=False)
                    nc.tensor.matmul(ycls[:], cst["c3lo"][:], x2hi[:], start=False, stop=False)
                    nc.tensor.matmul(ycls[:], cst["c3hi"][:], x2lo[:], start=False, stop=True)
                    # cls + b3c in fp32 on DVE (exact bias add)
                    cls_sb = sp.tile([128, T], F32, tag="cls")
                    nc.vector.tensor_scalar(out=cls_sb[:], in0=ycls[:],
                                            scalar1=cst["b3c"][:], scalar2=None,
                                            op0=mybir.AluOpType.add)
                    # ---- mask row -> pm row slot (column-selector lhsT)
                    slot = (b % 4) * NTILES + i
                    nc.tensor.matmul(pm, cst["wm16s"][:, 16 * slot:16 * slot + 16],
                                     x2hi[:], start=(slot == 0), stop=(slot == 15),
                                     skip_group_check=True)
                    # ---- argmax: all-reduce max across partitions, exact compare
                    maxbc = sp.tile([128, T], F32, tag="maxbc")
                    nc.gpsimd.partition_all_reduce(maxbc[:], cls_sb[:], channels=128,
                                                   reduce_op=bass_isa.ReduceOp.max)
                    onehot = sp.tile([128, T], F32R, tag="onehot")
                    nc.vector.tensor_tensor(onehot[:], cls_sb[:], maxbc[:],
                                            op=mybir.AluOpType.is_equal)
                    # ---- regression branch: r = lrelu(Wr @ x + br) -> fp8 tok half 0
                    pr = prh.tile([128, T], F32, tag="rh")
                    nc.tensor.matmul(pr[:], cst["wr"][:], xhi[:])
                    tok = ap.tile([128, 2 * T], FP8, tag="tok")
                    nc.scalar.activation(tok[:, 0:T], pr[:], mybir.ActivationFunctionType.Lrelu,
                                         bias=cst["br"][:], scale=1.0, alpha=SLOPE)
                    # tok half 1 = h1 in fp8 (gpsimd copy from the f32r rounding)
                    nc.gpsimd.tensor_copy(tok[:, T:2 * T], h1hi[:].bitcast(F32))
                    tok3 = tok[:].rearrange("p (two t) -> p two t", two=2)
                    # ---- CondMul1 for all 8 superclasses (fp8 DoubleRow)
                    hdr = ap.tile([128, 2 * T], FP8, tag="hdr")
                    pha = prh.tile([128, T], F32, tag="rh")
                    nc.tensor.matmul(pha[:], cst["w2dra"][:], tok3,
                                     perf_mode=mybir.MatmulPerfMode.DoubleRow)
                    nc.scalar.activation(hdr[:, 0:T], pha[:], mybir.ActivationFunctionType.Lrelu,
                                         bias=cst["b2a"][:], scale=1.0, alpha=SLOPE)
                    phb = prh.tile([128, T], F32, tag="rh")
                    nc.tensor.matmul(phb[:], cst["w2drb"][:], tok3,
                                     perf_mode=mybir.MatmulPerfMode.DoubleRow)
                    nc.scalar.activation(hdr[:, T:2 * T], phb[:], mybir.ActivationFunctionType.Lrelu,
                                         bias=cst["b2b"][:], scale=1.0, alpha=SLOPE)
                    # ---- CondMul2 for all classes (fp8 DoubleRow, unscaled)
                    hdr3 = hdr[:].rearrange("p (two t) -> p two t", two=2)
                    preg = pcr.tile([128, T], F32, tag="cr")
                    nc.tensor.matmul(preg[:], cst["w3xdr"][:], hdr3,
                                     perf_mode=mybir.MatmulPerfMode.DoubleRow)
                    # ---- G = onehot * (reg_all/128 + (b3+c)/128); ones^T G -> x_real
                    g = sp.tile([128, T], F32R, tag="g")
                    nc.vector.scalar_tensor_tensor(g[:], in0=preg[:], scalar=cst["b3iota"][:],
                                                   in1=onehot[:].bitcast(F32),
                                                   op0=mybir.AluOpType.add,
                                                   op1=mybir.AluOpType.mult)
                    nc.tensor.matmul(px, cst["ones16s"][:, 16 * slot:16 * slot + 16],
                                     g[:], start=(slot == 0), stop=(slot == 15),
                                     skip_group_check=True)
                # ---- per-group evac + store (ACT adds bm before lrelu)
                if b % 4 == 3:
                    mk_sb = op_.tile([16, T], F32, tag="mk")
                    nc.scalar.activation(mk_sb[:], pm, mybir.ActivationFunctionType.Lrelu,
                                         bias=cst["bm16"][:], scale=1.0, alpha=SLOPE)
                    nc.sync.dma_start(mkv[b // 4], mk_sb[:])
                    xr_sb = op_.tile([16, T], F32, tag="xr")
                    nc.vector.tensor_copy(xr_sb[:], px)
                    nc.sync.dma_start(xrv[b // 4], xr_sb[:])

    nc.compile()
    return nc


_CACHE = {}


def kernel(x_in, cl1_w, cl1_b, cl2_w, cl2_b, cl3_w, cl3_b,
           reg1_w, reg1_b, w2, b2, w3, b3):
    if "nc" not in _CACHE:
        _CACHE["nc"] = build_nc()
    nc = _CACHE["nc"]

    consts = prepare_consts(cl1_w, cl1_b, cl2_w, cl2_b, cl3_w, cl3_b,
                            reg1_w, reg1_b, w2, b2, w3, b3)
    x_in = np.ascontiguousarray(np.asarray(x_in, np.float32))
    xhi = _round_f32r(x_in)
    xlo = _round_f32r(x_in - xhi)
    in_maps = []
    for core in range(N_CORES):
        sl = slice(core * BS, (core + 1) * BS)
        m = {"xhi": np.ascontiguousarray(xhi[sl]),
             "xlo": np.ascontiguousarray(xlo[sl])}
        m.update(consts)
        in_maps.append(m)

    res = bass_utils.run_bass_kernel_spmd(nc, in_maps, core_ids=list(range(N_CORES)))
    x_real = np.concatenate([r["x_real"] for r in res.results], axis=0)
    mask = np.concatenate([r["mask"] for r in res.results], axis=0)
    return x_real, mask

